# revision 1
# baseline (speedup 1.0000x reference)
"""Causal GQA attention (S=2048, B=2, HQ=32, HKV=8, D=128) on 8 trn2 cores.

Sharding: the 16 (batch, kv-head) pairs are split 2 per core (data+head
parallel). Each pair carries group=4 query heads -> 8 attention heads/core.

Device kernel computes, per head, S^T = (Q K^T)^T in PSUM chunk-by-chunk
(so the softmax free axis never needs an on-chip transpose), exponentiates
on ACT into SBUF (P^T), applies the causal triangular mask only on the
128x128 diagonal block, then accumulates out^T = V^T-style matmuls with V
stationary and the softmax denominators with a ones-column matmul. All
matmul operands are viewed as float32r (full-rate fp32 on the PE array for
moving dim >= 256).

Host side only re-lays-out data: Q/K are fed pre-transposed [d, s], V as
[k_local, ktile, d], and the returned out^T [d, s] is transposed back.
"""

import numpy as np

import concourse.bass as bass
import concourse.mybir as mybir
import concourse.tile as tile
from concourse import bacc, bass_utils
from concourse.masks import make_upper_triangular

S, B, HQ, HKV, D = 2048, 2, 32, 8, 128
G = HQ // HKV                      # 4 query heads per kv head
NCORES = 8
NPAIRS = B * HKV                   # 16 (batch, kv-head) pairs
PAIRS_PER_CORE = NPAIRS // NCORES  # 2
HEADS_PER_CORE = PAIRS_PER_CORE * G  # 8
SCALE = 1.0 / float(np.sqrt(D))
QC = 512                           # q-chunk (PSUM bank) width
NQC = S // QC                      # 4
KT = 128                           # k-tile (partition) width
NKT = S // KT                      # 16

F32 = mybir.dt.float32
F32R = mybir.dt.float32r
BF16 = mybir.dt.bfloat16


def emit_core_program(tc, qt, kt, v, recd, ot):
    """Emit the per-core program.

    qt: [HEADS_PER_CORE, D, S] f32r   Q^T per head ([d, q])
    kt: [PAIRS_PER_CORE, D, S] f32r   K^T per pair ([d, k])
    v:  [PAIRS_PER_CORE, 128, NKT*D] f32  V per pair ([k_local, kt, d])
    recd: [HEADS_PER_CORE, NQC, QC] f32 DRAM scratch for 1/sum rows
    ot: [HEADS_PER_CORE, D, S] f32   out^T per head ([d, q])

    QK^T runs in float32r (full-rate fp32); the P*V side runs in bf16
    (P in [0, e^~5], V order-1: bf16 keeps ~4e-3 relative accuracy and the
    softmax normalization cancels much of the P rounding).
    """
    from contextlib import ExitStack

    nc = tc.nc
    with ExitStack() as ctx:
        _emit_core_program(ctx, tc, nc, qt, kt, v, recd, ot)


def _emit_core_program(ctx, tc, nc, qt, kt, v, recd, ot):
    singles = ctx.enter_context(tc.tile_pool(name="singles", bufs=1))
    kv_pool = ctx.enter_context(tc.tile_pool(name="kv", bufs=2))
    q_pool = ctx.enter_context(tc.tile_pool(name="q", bufs=2))
    pt_pool = ctx.enter_context(tc.tile_pool(name="pt", bufs=3))
    ob_pool = ctx.enter_context(tc.tile_pool(name="ob", bufs=3))
    nrm_pool = ctx.enter_context(tc.tile_pool(name="nrm", bufs=3))
    ps_s = ctx.enter_context(tc.tile_pool(name="ps_s", bufs=1, space="PSUM"))
    ps_o = ctx.enter_context(tc.tile_pool(name="ps_o", bufs=5, space="PSUM"))
    ps_sum = ctx.enter_context(tc.tile_pool(name="ps_sum", bufs=1, space="PSUM"))

    # Constants
    # tri[k, q] = 1.0 where q >= k (allowed), 0.0 where q < k. Multiplied
    # into the P^T diagonal block after exp (bf16, off the QK->exp path).
    trif = singles.tile([128, 128], F32)
    make_upper_triangular(nc, trif[:], val=1.0, diag=True)
    tri = singles.tile([128, 128], BF16)
    nc.scalar.copy(out=tri[:], in_=trif[:])
    onesc = singles.tile([128, 1], BF16)   # ones column (sum-over-k lhsT)
    nc.vector.memset(onesc[:], 1.0)

    for pair in range(PAIRS_PER_CORE):
        kt_sb = kv_pool.tile([D, S], F32R, tag="kt")
        nc.sync.dma_start(out=kt_sb[:], in_=kt[pair])
        v_sb = kv_pool.tile([128, NKT * D], BF16, tag="v")
        nc.gpsimd.dma_start(out=v_sb[:], in_=v[pair])  # casting DMA f32->bf16

        for g in range(G):
            head = pair * G + g
            q_sb = q_pool.tile([D, S], F32R)
            nc.sync.dma_start(out=q_sb[:], in_=qt[head])

            s_ps = ps_s.tile([128, 2 * QC], F32)    # 2 banks of S^T staging
            # out^T accumulators: one PSUM bank per q-chunk, rotating through
            # 5 banks so the next head's chunk never WARs on this head's
            # in-flight normalization
            o_tiles = [ps_o.tile([128, QC], F32, tag="o", name=f"o_{head}_{c}")
                       for c in range(NQC)]
            sum_ps = ps_sum.tile([128, QC], F32)    # 1 bank: chunk c at row 32c

            norm_state = {}

            def norm_stage_a(c):
                # sums row PSUM -> SBUF (ACT), then DMA-reshape to [128, 4]
                # so the reciprocal runs 128 lanes wide
                row = slice(32 * c, 32 * c + 1)
                sr = nrm_pool.tile([128, QC], F32, tag="sumrow")
                nc.scalar.copy(out=sr[row, :], in_=sum_ps[row, :])
                srec = nrm_pool.tile([128, NQC], F32, tag="srec")
                nc.sync.dma_start(out=srec[:], in_=sr[row, :])
                norm_state[c] = srec

            def norm_stage_b(c):
                srec = norm_state[c]
                srec2 = nrm_pool.tile([128, NQC], F32, tag="srec2")
                nc.vector.reciprocal(out=srec2[:], in_=srec[:])
                nc.sync.dma_start(out=recd[head, c], in_=srec2[:])
                bcs = nrm_pool.tile([128, QC], F32, tag="bc")
                nc.sync.dma_start(
                    out=bcs[:], in_=recd[head, c].partition_broadcast(128))
                norm_state[c] = bcs

            def norm_stage_c(c):
                bcs = norm_state.pop(c)
                osb = ob_pool.tile([128, QC], F32)
                nc.vector.tensor_mul(osb[:], o_tiles[c][:], bcs[:])
                nc.sync.dma_start(
                    out=ot[head][:, QC * c:QC * (c + 1)], in_=osb[:])

            for kti in range(NKT):
                w = KT * kti          # first allowed q for this k-tile
                c0 = w // QC          # first overlapping q-chunk
                p_kt = pt_pool.tile([128, S], BF16)  # P^T rows for this k-tile

                def s_slice(c):
                    off = max(0, w - QC * c)
                    base = QC * ((c - c0) % 2)
                    return off, s_ps[:, base + off:base + QC]

                def av_ones(c):
                    off = max(0, w - QC * c)
                    rhs = p_kt[:, QC * c + off:QC * (c + 1)]
                    first = kti == 0
                    last = kti == 4 * c + 3
                    nc.tensor.matmul(
                        out=o_tiles[c][:, off:QC],
                        lhsT=v_sb[:, D * kti:D * (kti + 1)],
                        rhs=rhs, start=first, stop=last,
                    )
                    nc.tensor.matmul(
                        out=sum_ps[32 * c:32 * c + 1, off:QC],
                        lhsT=onesc[:],
                        rhs=rhs, start=first, stop=last,
                        tile_position=(0, 32 * c),
                    )

                # interleave QK -> exp -> (prev chunk AV) so PE always has a
                # runnable matmul while ACT exponentiates
                prev = None
                for c in range(c0, NQC):
                    off, s_ap = s_slice(c)
                    nc.tensor.matmul(
                        out=s_ap,
                        lhsT=kt_sb[:, w:w + KT],
                        rhs=q_sb[:, QC * c + off:QC * (c + 1)],
                        start=True, stop=True,
                    )
                    nc.scalar.activation(
                        p_kt[:, QC * c + off:QC * (c + 1)], s_ap,
                        mybir.ActivationFunctionType.Exp, scale=SCALE)
                    if c == c0:
                        # causal mask: zero q < k on the diagonal block
                        nc.vector.tensor_mul(
                            p_kt[:, w:w + KT], p_kt[:, w:w + KT], tri[:])
                    if prev is not None:
                        av_ones(prev)
                    prev = c
                av_ones(prev)

                # Normalization, software-pipelined across k-tile iterations
                # so the slow partition-broadcast DMA never blocks the DVE
                # stream: chunk c finishes accumulating at kti=4c+3 (stage A:
                # pull sums row + reshape), recip + broadcast issue at 4c+4
                # (stage B), multiply + store at 4c+5 (stage C).
                if kti >= 3 and (kti - 3) % 4 == 0:
                    norm_stage_a((kti - 3) // 4)
                if kti >= 4 and (kti - 4) % 4 == 0:
                    norm_stage_b((kti - 4) // 4)
                if kti >= 5 and (kti - 5) % 4 == 0:
                    norm_stage_c((kti - 5) // 4)

            # drain chunk 3 (finished at kti=15)
            norm_stage_b(3)
            norm_stage_c(3)


_CACHED_NC = None


def build_program():
    global _CACHED_NC
    if _CACHED_NC is not None:
        return _CACHED_NC
    nc = bacc.Bacc("TRN2", target_bir_lowering=False, debug=False,
                   num_devices=NCORES)
    qt = nc.dram_tensor("qt", [HEADS_PER_CORE, D, S], F32R,
                        kind="ExternalInput").ap()
    kt = nc.dram_tensor("kt", [PAIRS_PER_CORE, D, S], F32R,
                        kind="ExternalInput").ap()
    v = nc.dram_tensor("v", [PAIRS_PER_CORE, 128, NKT * D], F32,
                       kind="ExternalInput").ap()
    recd = nc.dram_tensor("recd", [HEADS_PER_CORE, NQC, QC], F32,
                          kind="Internal").ap()
    ot = nc.dram_tensor("ot", [HEADS_PER_CORE, D, S], F32,
                        kind="ExternalOutput").ap()
    with tile.TileContext(nc) as tc:
        emit_core_program(tc, qt, kt, v, recd, ot)
    nc.compile()
    _CACHED_NC = nc
    return nc


def shard_inputs(query, key, value):
    """Full inputs -> list of 8 per-core in_maps (host-side relayout only)."""
    query = np.asarray(query, dtype=np.float32)
    key = np.asarray(key, dtype=np.float32)
    value = np.asarray(value, dtype=np.float32)

    # Q: [S,B,HQ,D] -> [B*HKV, G, D, S]
    qtall = np.ascontiguousarray(
        query.reshape(S, B, HKV, G, D).transpose(1, 2, 3, 4, 0)
    ).reshape(NPAIRS, G, D, S)
    # K: [S,B,HKV,D] -> [B*HKV, D, S]
    ktall = np.ascontiguousarray(
        key.transpose(1, 2, 3, 0)).reshape(NPAIRS, D, S)
    # V: [S,B,HKV,D] -> [B*HKV, k_local=128, NKT*D]
    vall = np.ascontiguousarray(
        value.reshape(NKT, 128, B, HKV, D).transpose(2, 3, 1, 0, 4)
    ).reshape(NPAIRS, 128, NKT * D)

    in_maps = []
    for c in range(NCORES):
        p0 = PAIRS_PER_CORE * c
        p1 = p0 + PAIRS_PER_CORE
        in_maps.append({
            "qt": np.ascontiguousarray(qtall[p0:p1].reshape(HEADS_PER_CORE, D, S)),
            "kt": np.ascontiguousarray(ktall[p0:p1]),
            "v": np.ascontiguousarray(vall[p0:p1]),
        })
    return in_maps


def unshard_output(results):
    """8 per-core {'ot': [8, D, S]} -> full [S, B, HQ, D]."""
    ot = np.stack([r["ot"] for r in results])          # [8, 8, D, S]
    ot = ot.reshape(B, HKV, G, D, S)                   # pairs major -> b, hkv
    out = np.ascontiguousarray(ot.transpose(4, 0, 1, 2, 3))  # [S,B,HKV,G,D]
    return out.reshape(S, B, HQ, D)


def kernel(query, key, value, _trace=False, _return_bkr=False):
    nc = build_program()
    in_maps = shard_inputs(query, key, value)
    bkr = bass_utils.run_bass_kernel_spmd(
        nc, in_maps, core_ids=list(range(NCORES)), trace=_trace)
    out = unshard_output(bkr.results)
    if _return_bkr:
        return out, bkr
    return out


if __name__ == "__main__":
    q = np.random.randn(S, B, HQ, D).astype(np.float32)
    k = np.random.randn(S, B, HKV, D).astype(np.float32)
    vv = np.random.randn(S, B, HKV, D).astype(np.float32)
    o = kernel(q, k, vv)
    print("out", o.shape, o.dtype, float(np.abs(o).max()))



# revision 4
# speedup vs baseline: 2.4532x; 2.4532x over previous
"""Causal GQA attention (S=2048, B=2, HQ=32, HKV=8, D=128) on 8 trn2 cores.

Sharding: the 16 (batch, kv-head) pairs are split 2 per core (data+head
parallel). Each pair carries group=4 query heads -> 8 attention heads/core.

Device kernel per head computes S^T = (Q K^T)^T chunk-group by chunk-group
in PSUM (so the softmax free axis never needs an on-chip transpose),
exponentiates on ACT into SBUF (P^T, fp16), applies the causal triangular
mask on the 128x128 diagonal block (Pool engine), and accumulates
out^T = V^T P^T matmuls with V stationary. The softmax denominators are NOT
computed with PE ones-matmuls: instead the Vector engine accumulates
R = sum_ktile P^T (elementwise fp16 adds, 2-byte DVE fast path), R is DMA'd
out, and the host finishes denom = R.sum(partitions) and the divide. All
matmul operands are fp16 (1 col/cycle on the PE; more precise than bf16 at
these magnitudes), PSUM accumulation fp32.

PSUM budget/partition: 2 x [128,1024] f32 S^T staging (8KB) + 4 x [128,512]
f32 out accumulators (8KB) = 16KB exactly.

Host side only re-lays-out data: Q/K are fed pre-transposed [d, s], V as
[k_local, ktile, d]; the returned out^T [d, s] (unnormalized, fp16) is
divided by the denominators and transposed back.
"""

import numpy as np

import concourse.bass as bass
import concourse.mybir as mybir
import concourse.tile as tile
from concourse import bacc, bass_utils
from concourse.masks import make_upper_triangular

S, B, HQ, HKV, D = 2048, 2, 32, 8, 128
G = HQ // HKV                      # 4 query heads per kv head
NCORES = 8
NPAIRS = B * HKV                   # 16 (batch, kv-head) pairs
PAIRS_PER_CORE = NPAIRS // NCORES  # 2
HEADS_PER_CORE = PAIRS_PER_CORE * G  # 8
SCALE = 1.0 / float(np.sqrt(D))
QC = 512                           # q-chunk (PSUM bank) width
NQC = S // QC                      # 4
KT = 128                           # k-tile (partition) width
NKT = S // KT                      # 16

F32 = mybir.dt.float32
F16 = mybir.dt.float16


def head_groups():
    """Chunk groups per head: (kti, chunks, off).

    For k-tile kti (k rows [w, w+128)), live q columns are [w, S).  Chunks of
    512 q-columns c0..3 (c0 = kti//4) are processed in pairs so consecutive
    matmuls share the stationary operand and exp covers 1024 columns.  off is
    the causal offset inside the first chunk of the FIRST group only.
    """
    groups = []
    for kti in range(NKT):
        w = KT * kti
        c0 = w // QC
        off = w - QC * c0
        cs = list(range(c0, NQC))
        first = True
        while cs:
            take = cs[:2]
            cs = cs[2:]
            groups.append((kti, tuple(take), off if first else 0))
            first = False
    return groups


GROUPS = head_groups()  # 24 groups


def emit_core_program(tc, qt, kt, v, ot, rd):
    """qt: [8, D, S] f32 DRAM (Q^T per head), kt: [2, D, S] f32, v: [2, 128,
    NKT*D] f32, ot: [8, D, S] f16 out (unnormalized out^T), rd: [8, 128, S]
    f16 out (per-k-row partial exp sums; host reduces partitions)."""
    from contextlib import ExitStack

    nc = tc.nc
    with ExitStack() as ctx:
        _emit(ctx, tc, nc, qt, kt, v, ot, rd)


def _emit(ctx, tc, nc, qt, kt, v, ot, rd):
    singles = ctx.enter_context(tc.tile_pool(name="singles", bufs=1))
    kv_pool = ctx.enter_context(tc.tile_pool(name="kv", bufs=2))
    q_pool = ctx.enter_context(tc.tile_pool(name="q", bufs=2))
    p_pool = ctx.enter_context(tc.tile_pool(name="p", bufs=4))
    r_pool = ctx.enter_context(tc.tile_pool(name="r", bufs=2))
    ob_pool = ctx.enter_context(tc.tile_pool(name="ob", bufs=3))
    ps_s = ctx.enter_context(tc.tile_pool(name="ps_s", bufs=2, space="PSUM"))
    ps_o = ctx.enter_context(tc.tile_pool(name="ps_o", bufs=4, space="PSUM"))

    # tri[k, q] = 1.0 where q >= k (allowed), 0.0 where q < k; multiplied into
    # the P^T diagonal block after exp.
    trif = singles.tile([128, 128], F32)
    make_upper_triangular(nc, trif[:], val=1.0, diag=True)
    tri = singles.tile([128, 128], F16)
    nc.scalar.copy(out=tri[:], in_=trif[:])

    # One-group-deep software pipeline over the whole program: the AV matmuls
    # for group i are emitted after the QK matmuls of group i+1, so the PE
    # always has runnable work while ACT exponentiates group i+1.
    pending = []  # [(head_ctx, kti, cs, off, p_tile)]

    def flush_pending():
        hctx, kti, cs, off, p_t = pending.pop(0)
        v_sb_, o_tiles_, head_ = hctx["v_sb"], hctx["o"], hctx["head"]
        for j, c in enumerate(cs):
            o_c = off if j == 0 else 0
            t0 = QC * j + o_c
            nc.tensor.matmul(
                out=o_tiles_[c][:, o_c:QC],
                lhsT=v_sb_[:, D * kti:D * (kti + 1)],
                rhs=p_t[:, t0:QC * (j + 1)],
                start=(kti == 0), stop=(kti == 4 * c + 3),
            )
            if kti == 4 * c + 3:
                # chunk finished accumulating: drain PSUM -> SBUF f16 -> DRAM
                osb = ob_pool.tile([128, QC], F16, tag="osb",
                                   name=f"osb_{head_}_{c}")
                nc.vector.tensor_copy(out=osb[:], in_=o_tiles_[c][:])
                nc.sync.dma_start(
                    out=ot[head_][:, QC * c:QC * (c + 1)], in_=osb[:])

    # Input loads (all casting f32->f16 DMAs, so gpsimd-triggered) are
    # prefetched one head ahead so the triggers land on the gpsimd queue
    # before the current head's mask ops, not after them.
    pair_res = {}
    q_res = {}

    def load_pair(pr):
        kt_sb = kv_pool.tile([D, S], F16, tag="kt", name=f"kt_{pr}")
        nc.gpsimd.dma_start(out=kt_sb[:], in_=kt[pr])
        v_sb = kv_pool.tile([128, NKT * D], F16, tag="v", name=f"v_{pr}")
        nc.gpsimd.dma_start(out=v_sb[:], in_=v[pr])
        pair_res[pr] = (kt_sb, v_sb)

    def load_q(h):
        q_sb = q_pool.tile([D, S], F16, tag="q", name=f"q_{h}")
        nc.gpsimd.dma_start(out=q_sb[:], in_=qt[h])
        q_res[h] = q_sb

    load_pair(0)
    load_q(0)
    for head in range(HEADS_PER_CORE):
        pair = head // G
        if head + 1 < HEADS_PER_CORE:
            if (head + 1) // G != pair:
                load_pair((head + 1) // G)
            load_q(head + 1)
        if True:
            kt_sb, v_sb = pair_res[pair]
            q_sb = q_res.pop(head)
            r_sb = r_pool.tile([128, S], F16, tag="r", name=f"r_{head}")
            o_tiles = [ps_o.tile([128, QC], F32, tag="o", name=f"o_{head}_{c}")
                       for c in range(NQC)]
            hctx = {"v_sb": v_sb, "o": o_tiles, "head": head}

            for kti, cs, off in GROUPS:
                w = KT * kti
                ncols = QC * len(cs)
                s_t = ps_s.tile([128, 2 * QC], F32, tag="s",
                                name=f"s_{head}_{kti}_{cs[0]}")
                # QK^T: consecutive matmuls share lhsT (k-tile of K^T)
                for j, c in enumerate(cs):
                    o_c = off if j == 0 else 0
                    nc.tensor.matmul(
                        out=s_t[:, QC * j + o_c:QC * (j + 1)],
                        lhsT=kt_sb[:, w:w + KT],
                        rhs=q_sb[:, QC * c + o_c:QC * (c + 1)],
                        start=True, stop=True,
                    )
                # exp on ACT (one instruction for the whole group)
                p_t = p_pool.tile([128, 2 * QC], F16, tag="p",
                                  name=f"p_{head}_{kti}_{cs[0]}")
                nc.scalar.activation(
                    p_t[:, off:ncols], s_t[:, off:ncols],
                    mybir.ActivationFunctionType.Exp, scale=SCALE)
                # causal mask on the diagonal block (first group of each kti)
                if QC * cs[0] <= w < QC * (cs[0] + 1):
                    nc.gpsimd.tensor_mul(
                        p_t[:, off:off + KT], p_t[:, off:off + KT], tri[:])
                # denominator partials: R += P^T (Vector engine, fp16)
                rcol = slice(QC * cs[0] + off, QC * (cs[-1] + 1))
                if kti == 0:
                    nc.vector.tensor_copy(
                        out=r_sb[:, rcol], in_=p_t[:, off:ncols])
                else:
                    nc.vector.tensor_add(
                        r_sb[:, rcol], r_sb[:, rcol], p_t[:, off:ncols])

                pending.append((hctx, kti, cs, off, p_t))
                if len(pending) > 1:
                    flush_pending()

            # R is complete once the last group's add ran; DMA it out
            nc.sync.dma_start(out=rd[head], in_=r_sb[:])

    while pending:
        flush_pending()


_CACHED_NC = None


def build_program():
    global _CACHED_NC
    if _CACHED_NC is not None:
        return _CACHED_NC
    nc = bacc.Bacc("TRN2", target_bir_lowering=False, debug=False,
                   num_devices=NCORES)
    qt = nc.dram_tensor("qt", [HEADS_PER_CORE, D, S], F32,
                        kind="ExternalInput").ap()
    kt = nc.dram_tensor("kt", [PAIRS_PER_CORE, D, S], F32,
                        kind="ExternalInput").ap()
    v = nc.dram_tensor("v", [PAIRS_PER_CORE, 128, NKT * D], F32,
                       kind="ExternalInput").ap()
    ot = nc.dram_tensor("ot", [HEADS_PER_CORE, D, S], F16,
                        kind="ExternalOutput").ap()
    rd = nc.dram_tensor("rd", [HEADS_PER_CORE, 128, S], F16,
                        kind="ExternalOutput").ap()
    with tile.TileContext(nc) as tc:
        emit_core_program(tc, qt, kt, v, ot, rd)
    nc.compile()
    _CACHED_NC = nc
    return nc


def shard_inputs(query, key, value):
    """Full inputs -> list of 8 per-core in_maps (host-side relayout only)."""
    query = np.asarray(query, dtype=np.float32)
    key = np.asarray(key, dtype=np.float32)
    value = np.asarray(value, dtype=np.float32)

    # Q: [S,B,HQ,D] -> [B*HKV, G, D, S]
    qtall = np.ascontiguousarray(
        query.reshape(S, B, HKV, G, D).transpose(1, 2, 3, 4, 0)
    ).reshape(NPAIRS, G, D, S)
    # K: [S,B,HKV,D] -> [B*HKV, D, S]
    ktall = np.ascontiguousarray(
        key.transpose(1, 2, 3, 0)).reshape(NPAIRS, D, S)
    # V: [S,B,HKV,D] -> [B*HKV, k_local=128, NKT*D]
    vall = np.ascontiguousarray(
        value.reshape(NKT, 128, B, HKV, D).transpose(2, 3, 1, 0, 4)
    ).reshape(NPAIRS, 128, NKT * D)

    in_maps = []
    for c in range(NCORES):
        p0 = PAIRS_PER_CORE * c
        p1 = p0 + PAIRS_PER_CORE
        in_maps.append({
            "qt": np.ascontiguousarray(qtall[p0:p1].reshape(HEADS_PER_CORE, D, S)),
            "kt": np.ascontiguousarray(ktall[p0:p1]),
            "v": np.ascontiguousarray(vall[p0:p1]),
        })
    return in_maps


def unshard_output(results):
    """8 per-core {'ot','rd'} -> full [S, B, HQ, D] (normalize on host)."""
    ot = np.stack([np.asarray(r["ot"], dtype=np.float32) for r in results])
    rd = np.stack([np.asarray(r["rd"], dtype=np.float32) for r in results])
    denom = rd.sum(axis=2)                         # [8, 8, S]
    ot /= denom[:, :, None, :]                     # [8, 8, D, S]
    ot = ot.reshape(B, HKV, G, D, S)               # pairs major -> b, hkv
    out = np.ascontiguousarray(ot.transpose(4, 0, 1, 2, 3))  # [S,B,HKV,G,D]
    return out.reshape(S, B, HQ, D)


def kernel(query, key, value, _trace=False, _return_bkr=False):
    nc = build_program()
    in_maps = shard_inputs(query, key, value)
    bkr = bass_utils.run_bass_kernel_spmd(
        nc, in_maps, core_ids=list(range(NCORES)), trace=_trace)
    out = unshard_output(bkr.results)
    if _return_bkr:
        return out, bkr
    return out


if __name__ == "__main__":
    q = np.random.randn(S, B, HQ, D).astype(np.float32)
    k = np.random.randn(S, B, HKV, D).astype(np.float32)
    vv = np.random.randn(S, B, HKV, D).astype(np.float32)
    o = kernel(q, k, vv)
    print("out", o.shape, o.dtype, float(np.abs(o).max()))
